# revision 26
# baseline (speedup 1.0000x reference)
"""Single-head attention (B=4, S=2048, D=1024) on 8 Trainium2 NeuronCores.

Sharding: batch x KEY-half with a pairwise Q exchange. Core c handles batch
b=c//2 and token half h=c%2 (its 1024 "own" tokens are both its keys and the
first half of its query order). Each core computes K/V/Q for its OWN 1024
tokens only (no duplicated projection work), then the two cores of a batch
exchange Q via an in-kernel AllGather (replica groups {0,1},{2,3},...). Each
core then computes the UNNORMALIZED partial attention O~ = exp(S)V for ALL
2048 queries against its 1024 keys, plus partial row-sums; the host combines
the pair: O = (O~_0 + O~_1) / (rs_0 + rs_1).

The AllGather output is rank-ordered ([Q_even_core, Q_odd_core]) — identical
on both cores — but each core needs the PARTNER half. To stay SPMD (no
per-core addressing), the partner half is reconstructed exactly as
qoth = (qx0 + qx1) - qown on DVE with an fp32 intermediate (a sum of two
bf16 is exact in fp32, so subtracting qown returns the partner's bf16 bits).
Queries are processed own-half-first ("rolled" order), so the first half of
phase C overlaps the collective; the host un-rolls odd cores' outputs.

All matmul operands are bf16 (PSUM accumulation stays fp32); per-core PE work
is 15.0 GFLOP (vs 17.2 without the exchange). Every bulk tensor is
host-arranged partition-major so it loads as ONE DMA with 16KB contiguous
lines per partition (the 2KB-line per-ec layout was descriptor-rate-bound
and starved phase Q). A dummy-matmul warmup keeps the PE busy through the
initial DMA window so the HAM clock gate reaches 8/8 before the first real
chain.

Per-core pipeline (activations kept [feature, token] transposed so the PE
contracts over partitions):
  Q:  Q^T[ec] = Wq[:,ec-blk].T @ x^T  (own 1024 tokens; DMA'd out as one
      block, AllGather'd during phases K/V)
  K:  K^T[ec] = Wk[:,ec-blk].T @ x^T
  V:  V[kc]   = x^T[:, kc-blk].T @ Wv  (bias via host-sent broadcast tile)
  C:  per 512-query block: S^T[k,q] = K^T.T @ Q^T; exp on ACT writes P^T
      straight to SBUF as bf16; partial row-sums via ones-vector matmuls
      fused into the O chain; O~ = P^T.T @ V; DMA out raw.
"""

import sys
from contextlib import ExitStack

import ml_dtypes
import numpy as np

if "/opt/trn_rl_repo" not in sys.path:
    sys.path.insert(0, "/opt/trn_rl_repo")

import concourse.bass as bass
import concourse.bacc as bacc
import concourse.tile as tile
from concourse import mybir
from concourse.bass_utils import run_bass_kernel_spmd

P = 128
S = 2048        # full sequence (queries per core)
SK = 1024       # own tokens per core (keys; also own query half)
D = 1024        # model dim
F32 = mybir.dt.float32
BF16 = mybir.dt.bfloat16
NPBF16 = ml_dtypes.bfloat16

DC = D // P     # 8 d-chunks (contraction over model dim)
EC = D // P     # 8 e-chunks (output features)
KC = SK // P    # 8 key chunks (own half)
NT = 512        # moving-operand tile (one PSUM bank of fp32)
QT = 512        # query tile in phase C
SB = SK // NT   # 2 token blocks per core
WARM = 16       # PE warm-up matmuls (cover the initial DMA window)

SCALE = 1.0 / float(np.sqrt(np.float32(D)))
Identity = mybir.ActivationFunctionType.Identity
Exp = mybir.ActivationFunctionType.Exp


def build_program() -> bass.Bass:
    nc = bacc.Bacc(
        "TRN2", target_bir_lowering=False, debug=False, num_devices=8)

    xT_d = nc.dram_tensor("xT", [SB, P, DC, NT], BF16,
                      kind="ExternalInput").ap()
    wq_d = nc.dram_tensor("Wqr", [EC, P, DC, P], BF16, kind="ExternalInput").ap()
    wk_d = nc.dram_tensor("Wkr", [EC, P, DC, P], BF16, kind="ExternalInput").ap()
    wv_d = nc.dram_tensor("Wvr", [P, DC, D], BF16, kind="ExternalInput").ap()
    bvx_d = nc.dram_tensor("bvxr", [P, D + 2 * EC], F32,
                       kind="ExternalInput").ap()
    o_d = nc.dram_tensor("o_raw", [S, D], BF16, kind="ExternalOutput").ap()
    rs_d = nc.dram_tensor("rs_raw", [S // QT, P, QT // P], F32,
                      kind="ExternalOutput").ap()

    with tile.TileContext(nc) as tc, ExitStack() as ctx:
        const_p = ctx.enter_context(tc.tile_pool(name="const", bufs=1))
        xt_p = ctx.enter_context(tc.tile_pool(name="xt", bufs=SB))
        kt_p = ctx.enter_context(tc.tile_pool(name="kt", bufs=EC))
        qo_p = ctx.enter_context(tc.tile_pool(name="qo", bufs=1))
        qg_p = ctx.enter_context(tc.tile_pool(name="qg", bufs=2))
        tq_p = ctx.enter_context(tc.tile_pool(name="tq", bufs=2))
        v_p = ctx.enter_context(tc.tile_pool(name="v", bufs=KC))
        wq_p = ctx.enter_context(tc.tile_pool(name="wq", bufs=EC))
        wk_p = ctx.enter_context(tc.tile_pool(name="wk", bufs=EC))
        w_p = ctx.enter_context(tc.tile_pool(name="w", bufs=1))
        pt_p = ctx.enter_context(tc.tile_pool(name="ptp", bufs=12))
        osb_p = ctx.enter_context(tc.tile_pool(name="osb", bufs=3))
        st_p = ctx.enter_context(tc.tile_pool(name="stat", bufs=2))
        psA = ctx.enter_context(tc.tile_pool(name="psA", bufs=4, space="PSUM"))
        psB = ctx.enter_context(tc.tile_pool(name="psB", bufs=3, space="PSUM"))
        psR = ctx.enter_context(tc.tile_pool(name="psR", bufs=1, space="PSUM"))
        dram = ctx.enter_context(tc.tile_pool(name="dram", bufs=2,
                                              space="DRAM"))

        qxi = dram.tile([P, EC, SK], BF16)       # own Q^T, collective input
        qxo = dram.tile([2, P, EC, SK], BF16)    # gathered [even, odd]

        # ---- DMA issue order --------------------------------------------
        # Aggregate input bandwidth is ~240GB/s regardless of descriptor
        # shape, so what matters is ARRIVAL ORDER matching consumption
        # order (per-ec weight tiles keep the dependencies fine-grained).
        # Deadline order: xt[0]+wq[0] gate the first chain (~14us); then
        # one wq tile every ~3.4us; xt[1] by ~40us (qb=1 chains); wk by
        # 42..66us; wv/bvb by ~69us.
        xt = [xt_p.tile([P, DC, NT], BF16, name=f"xt{tb}", tag="xt")
              for tb in range(SB)]
        wq = [wq_p.tile([P, DC, P], BF16, name=f"wq{ec}", tag="wq")
              for ec in range(EC)]
        wk = [wk_p.tile([P, DC, P], BF16, name=f"wk{ec}", tag="wk")
              for ec in range(EC)]
        wv = w_p.tile([P, DC, D], BF16, name="wv", tag="w")
        ones_col = const_p.tile([P, 1], BF16)   # lhsT for row-sums
        nc.vector.memset(ones_col[:], 1.0)
        # bv broadcast + bq/bk biases in one fat-lined tensor (a separate
        # [P,16] bias DMA has 64B lines and poisons the queue for ~8us)
        bvx = const_p.tile([P, D + 2 * EC], F32)
        bvb = bvx[:, 0:D]

        # Three queues (sync/scalar HWDGE + gpsimd SWDGE) deliver in
        # deadline order — the DMA engines are chip-shared across all 8
        # cores, so each extra queue buys a real share of bandwidth. Lines
        # stay >= 2KB (smaller is descriptor-rate-bound and poisons the
        # queue). The qxi exchange DMA and collective ride behind gpsimd's
        # stream (firing when Q drains ~40us).
        T1, T2 = 43, 86  # partition thirds
        nc.sync.dma_start(xt[0][0:T1], xT_d[0][0:T1])
        nc.sync.dma_start(wq[0][:], wq_d[0])
        nc.sync.dma_start(wq[3][:], wq_d[3])
        nc.sync.dma_start(xt[1][0:T1], xT_d[1][0:T1])
        nc.sync.dma_start(wq[6][:], wq_d[6])
        for ec in (0, 3, 6):
            nc.sync.dma_start(wk[ec][:], wk_d[ec])
        nc.sync.dma_start(wv[0:P // 2], wv_d[0:P // 2])

        nc.scalar.dma_start(xt[0][T1:T2], xT_d[0][T1:T2])
        nc.scalar.dma_start(wq[1][:], wq_d[1])
        nc.scalar.dma_start(bvx[:], bvx_d[:])
        nc.scalar.dma_start(wq[4][:], wq_d[4])
        nc.scalar.dma_start(xt[1][T1:T2], xT_d[1][T1:T2])
        nc.scalar.dma_start(wq[7][:], wq_d[7])
        for ec in (1, 4, 7):
            nc.scalar.dma_start(wk[ec][:], wk_d[ec])
        nc.scalar.dma_start(wv[P // 2:P], wv_d[P // 2:P])

        nc.gpsimd.dma_start(xt[0][T2:P], xT_d[0][T2:P])
        nc.gpsimd.dma_start(wq[2][:], wq_d[2])
        nc.gpsimd.dma_start(wq[5][:], wq_d[5])
        nc.gpsimd.dma_start(xt[1][T2:P], xT_d[1][T2:P])
        for ec in (2, 5):
            nc.gpsimd.dma_start(wk[ec][:], wk_d[ec])

        # PE warm-up during the initial DMA window: dummy matmuls on a
        # memset scratch tile get the HAM clock gate to K=8/8 before the
        # first real chain. Every PSUM group gets a reader (narrow copy) —
        # matmul groups with no consumer have wedged the device.
        scr = const_p.tile([P, NT], BF16)
        nc.vector.memset(scr[:], 0.0)
        scr_out = const_p.tile([P, NT], F32)
        for i in range(WARM):
            pool = psA if i % 2 == 0 else psB
            ps = pool.tile([P, NT], F32)
            nc.tensor.matmul(
                ps[:], scr[:, 0:P], scr[:], start=True, stop=True)
            j = i % 8
            nc.scalar.activation(
                scr_out[:, j * 64:(j + 1) * 64], ps[:, 0:64], Identity)

        # ---- Phase Q: Q^T (own tokens) + exchange ------------------------
        # qb OUTER: the first chain needs only xt[0]+wq[0] (1.25MB), and
        # each later chain adds one 256KB wq tile — matches the DMA stream.
        qown = qo_p.tile([P, EC, SK], BF16, name="qown", tag="qo")
        for qb in range(SB):
            for ec in range(EC):
                ps = psA.tile([P, NT], F32)
                for dc in range(DC):
                    nc.tensor.matmul(
                        ps[:],
                        wq[ec][:, dc, :],
                        xt[qb][:, dc, :],
                        start=(dc == 0), stop=(dc == DC - 1),
                    )
                nc.scalar.activation(
                    qown[:, ec, qb * NT:(qb + 1) * NT], ps[:], Identity,
                    bias=bvx[:, D + ec:D + ec + 1],
                )
        nc.gpsimd.dma_start(qxi[:], qown[:])

        nc.gpsimd.collective_compute(
            "AllGather",
            mybir.AluOpType.bypass,
            replica_groups=[[0, 1], [2, 3], [4, 5], [6, 7]],
            ins=[qxi.opt()],
            outs=[qxo.opt()],
        )

        # ---- Phase K: K^T (own keys) resident ---------------------------
        kt = [kt_p.tile([P, SK], BF16, name=f"kt{ec}", tag="kt")
              for ec in range(EC)]
        for kb in range(SB):
            for ec in range(EC):
                ps = psA.tile([P, NT], F32)
                for dc in range(DC):
                    nc.tensor.matmul(
                        ps[:],
                        wk[ec][:, dc, :],
                        xt[kb][:, dc, :],
                        start=(dc == 0), stop=(dc == DC - 1),
                    )
                nc.scalar.activation(
                    kt[ec][:, kb * NT:(kb + 1) * NT], ps[:], Identity,
                    bias=bvx[:, D + EC + ec:D + EC + ec + 1],
                )

        # ---- Phase V: V natural [k, e] (own keys) resident --------------
        v = [v_p.tile([P, D], BF16, name=f"v{kc}", tag="v") for kc in range(KC)]
        for kc in range(KC):
            for et in range(D // NT):
                ps = psB.tile([P, NT], F32)
                tb, j = divmod(kc, NT // P)
                for dc in range(DC):
                    nc.tensor.matmul(
                        ps[:],
                        xt[tb][:, dc, j * P:(j + 1) * P],
                        wv[:, dc, et * NT:(et + 1) * NT],
                        start=(dc == 0), stop=(dc == DC - 1),
                    )
                # fused bias add on evacuation: v = ps + bvb
                nc.vector.scalar_tensor_tensor(
                    v[kc][:, et * NT:(et + 1) * NT], ps[:], 1.0,
                    bvb[:, et * NT:(et + 1) * NT],
                    mybir.AluOpType.mult, mybir.AluOpType.add,
                )

        # ---- Gather readback --------------------------------------------
        # qx0/qx1 = the two rank slots (identical on both cores). Partner
        # half = (qx0 + qx1) - qown with an fp32 intermediate, so the
        # reconstruction is bit-exact (sum of two bf16 is exact in fp32).
        # Readback rides gpsimd (slot 0) and sync (slot 1); the DVE
        # reconstruct is emitted mid-phase-C (after qq=0's evacuations) so
        # a late collective can't head-of-line-block the O evacuations.
        qx0 = qg_p.tile([P, EC, SK], BF16, name="qx0", tag="qg")
        qx1 = qg_p.tile([P, EC, SK], BF16, name="qx1", tag="qg")
        nc.gpsimd.dma_start(qx0[:], qxo[0])
        nc.sync.dma_start(qx1[:], qxo[1])
        qoth = qx1  # reconstructed in place

        # ---- Phase C: attention, transposed scores ----------------------
        # Query blocks in "rolled" order: qq 0-1 = own tokens (overlap the
        # collective), qq 2-3 = partner tokens via qoth.
        for qq in range(S // QT):
            if qq == 1:
                # exact partner reconstruct on DVE, off the critical path
                for ec in range(EC):
                    tmp = tq_p.tile([P, SK], F32, name="tq", tag="tq")
                    nc.vector.tensor_tensor(
                        tmp[:], qx0[:, ec, :], qx1[:, ec, :],
                        mybir.AluOpType.add)
                    nc.vector.tensor_tensor(
                        qoth[:, ec, :], tmp[:], qown[:, ec, :],
                        mybir.AluOpType.subtract)
            if qq < 2:
                qsrc, qo0 = qown, qq * QT
            else:
                qsrc, qo0 = qoth, (qq - 2) * QT
            # S^T[k, q] per key chunk; exp writes P^T straight to SBUF bf16
            ptt = [pt_p.tile([P, QT], BF16, tag="ptp", name=f"ptt{kc}")
                   for kc in range(KC)]
            for kc in range(KC):
                ps = psA.tile([P, QT], F32)
                for ec in range(EC):
                    nc.tensor.matmul(
                        ps[:],
                        kt[ec][:, kc * P:(kc + 1) * P],
                        qsrc[:, ec, qo0:qo0 + QT],
                        start=(ec == 0), stop=(ec == EC - 1),
                    )
                nc.scalar.activation(ptt[kc][:], ps[:], Exp, scale=SCALE)

            # O~ = P^T.T @ V per 128-query chunk, with the partial row-sum
            # fused in: after each et=0 matmul, an N=1 matmul with the SAME
            # stationary P^T block and a moving ones-column accumulates
            # rs[q] — its LDWEIGHTS hides under the 512-wide stream.
            rs_cols = st_p.tile([P, QT // P], F32, name="rsc", tag="rs")
            last_qq = qq == S // QT - 1
            for qc in range(QT // P):
                last_blk = last_qq and qc == QT // P - 1
                o_sb = osb_p.tile([P, D], BF16, name="osb", tag="osb")
                psq = psR.tile([P, 1], F32, name="psq")
                row0 = qq * QT + qc * P
                for et in range(D // NT):
                    ps = psB.tile([P, NT], F32)
                    for kc in range(KC):
                        nc.tensor.matmul(
                            ps[:],
                            ptt[kc][:, qc * P:(qc + 1) * P],
                            v[kc][:, et * NT:(et + 1) * NT],
                            start=(kc == 0), stop=(kc == KC - 1),
                        )
                        if et == 0:
                            nc.tensor.matmul(
                                psq[:],
                                ptt[kc][:, qc * P:(qc + 1) * P],
                                ones_col[:],
                                start=(kc == 0), stop=(kc == KC - 1),
                            )
                    nc.vector.tensor_copy(
                        o_sb[:, et * NT:(et + 1) * NT], ps[:])
                    if last_blk:
                        # drain the kernel tail: ship each half as soon as
                        # it's evacuated, with the tiny rs DMA in between
                        if et == 0:
                            nc.vector.tensor_copy(
                                rs_cols[:, qc:qc + 1], psq[:])
                            nc.sync.dma_start(
                                o_d[row0:row0 + P, 0:NT], o_sb[:, 0:NT])
                            nc.sync.dma_start(rs_d[qq], rs_cols[:])
                        else:
                            nc.sync.dma_start(
                                o_d[row0:row0 + P, NT:D], o_sb[:, NT:D])
                if not last_blk:
                    nc.vector.tensor_copy(rs_cols[:, qc:qc + 1], psq[:])
                    nc.sync.dma_start(o_d[row0:row0 + P, :], o_sb[:])
            if not last_qq:
                nc.sync.dma_start(rs_d[qq], rs_cols[:])

    nc.compile()
    return nc


_CACHE: dict = {}


def _get_program() -> bass.Bass:
    if "nc" not in _CACHE:
        _CACHE["nc"] = build_program()
    return _CACHE["nc"]


def kernel(x, Wq, bq, Wk, bk, Wv, bv, _trace=False, _trace_kwargs=None):
    nc = _get_program()
    x = np.asarray(x, dtype=np.float32)

    def wrearr(w):
        # [d, e] -> [ec, p(d%128), dc, e%128] so each ec-block DMA is
        # contiguous with 2KB per partition line
        w = np.asarray(w, np.float32).astype(NPBF16)
        return np.ascontiguousarray(
            w.reshape(DC, P, EC, P).transpose(2, 1, 0, 3))

    bvx = np.concatenate([
        np.broadcast_to(np.asarray(bv, np.float32), (P, D)),
        np.asarray(bq, np.float32).reshape(EC, P).T,
        np.asarray(bk, np.float32).reshape(EC, P).T,
    ], axis=1)
    shared = {
        "Wqr": wrearr(Wq),
        "Wkr": wrearr(Wk),
        "Wvr": np.ascontiguousarray(
            np.asarray(Wv, np.float32).astype(NPBF16)
            .reshape(DC, P, D).transpose(1, 0, 2)),
        "bvxr": np.ascontiguousarray(bvx),
    }
    in_maps = []
    for c in range(8):
        b, h = divmod(c, 2)
        xb = x[b][h * SK:(h + 1) * SK]      # own token half only
        xTb = xb.T.astype(NPBF16)           # [D, SK]
        # [d, s] -> [tb, p, dc, t]: each 512-token block fully contiguous
        xTr = np.ascontiguousarray(
            xTb.reshape(DC, P, SB, NT).transpose(2, 1, 0, 3))
        in_maps.append({"xT": xTr, **shared})

    res = run_bass_kernel_spmd(
        nc, in_maps, list(range(8)),
        trace=_trace, **(_trace_kwargs or {}),
    )
    out = np.empty((4, S, D), dtype=np.float32)
    for b in range(4):
        def unrs(r):
            # device writes rs as [qq, p, qc] = rowsum(qq*512 + qc*128 + p)
            return r.reshape(S // QT, P, QT // P).transpose(
                0, 2, 1).reshape(S).astype(np.float64)

        o0 = res.results[2 * b]["o_raw"].astype(np.float64)
        r0 = unrs(res.results[2 * b]["rs_raw"])
        o1 = res.results[2 * b + 1]["o_raw"].astype(np.float64)
        r1 = unrs(res.results[2 * b + 1]["rs_raw"])
        # core h=1 computed queries in rolled order; un-roll before combining
        o1 = np.roll(o1, SK, axis=0)
        r1 = np.roll(r1, SK)
        out[b] = ((o0 + o1) / (r0 + r1)[:, None]).astype(np.float32)
    if _trace:
        return out, res
    return out


# revision 28
# speedup vs baseline: 1.2621x; 1.2621x over previous
"""Single-head attention (B=4, S=2048, D=1024) on 8 Trainium2 NeuronCores.

Sharding: batch x KEY-half with a pairwise Q exchange. Core c handles batch
b=c//2 and token half h=c%2 (its 1024 "own" tokens are both its keys and the
first half of its query order). Each core computes K/V/Q for its OWN 1024
tokens only (no duplicated projection work), then the two cores of a batch
exchange Q via an in-kernel AllGather (replica groups {0,1},{2,3},...). Each
core then computes the UNNORMALIZED partial attention O~ = exp(S)V for ALL
2048 queries against its 1024 keys, plus partial row-sums; the host combines
the pair: O = (O~_0 + O~_1) / (rs_0 + rs_1).

The AllGather output is rank-ordered ([Q_even_core, Q_odd_core]) — identical
on both cores — but each core needs the PARTNER half. To stay SPMD (no
per-core addressing), the partner half is reconstructed exactly as
qoth = (qx0 + qx1) - qown on DVE with an fp32 intermediate (a sum of two
bf16 is exact in fp32, so subtracting qown returns the partner's bf16 bits).
Queries are processed own-half-first ("rolled" order), so the first half of
phase C overlaps the collective; the host un-rolls odd cores' outputs.

All matmul operands are bf16 (PSUM accumulation stays fp32); per-core PE work
is 15.0 GFLOP (vs 17.2 without the exchange). Every bulk tensor is
host-arranged partition-major so it loads as ONE DMA with 16KB contiguous
lines per partition (the 2KB-line per-ec layout was descriptor-rate-bound
and starved phase Q). A dummy-matmul warmup keeps the PE busy through the
initial DMA window so the HAM clock gate reaches 8/8 before the first real
chain.

Per-core pipeline (activations kept [feature, token] transposed so the PE
contracts over partitions):
  Q:  Q^T[ec] = Wq[:,ec-blk].T @ x^T  (own 1024 tokens; DMA'd out as one
      block, AllGather'd during phases K/V)
  K:  K^T[ec] = Wk[:,ec-blk].T @ x^T
  V:  V[kc]   = x^T[:, kc-blk].T @ Wv  (bias via host-sent broadcast tile)
  C:  per 512-query block: S^T[k,q] = K^T.T @ Q^T; exp on ACT writes P^T
      straight to SBUF as bf16; partial row-sums via ones-vector matmuls
      fused into the O chain; O~ = P^T.T @ V; DMA out raw.
"""

import sys
from contextlib import ExitStack

import ml_dtypes
import numpy as np

if "/opt/trn_rl_repo" not in sys.path:
    sys.path.insert(0, "/opt/trn_rl_repo")

import concourse.bass as bass
import concourse.bacc as bacc
import concourse.tile as tile
from concourse import mybir
from concourse.bass_utils import run_bass_kernel_spmd

P = 128
S = 2048        # full sequence (queries per core)
SK = 1024       # own tokens per core (keys; also own query half)
D = 1024        # model dim
F32 = mybir.dt.float32
BF16 = mybir.dt.bfloat16
NPBF16 = ml_dtypes.bfloat16

DC = D // P     # 8 d-chunks (contraction over model dim)
EC = D // P     # 8 e-chunks (output features)
KC = SK // P    # 8 key chunks (own half)
NT = 512        # moving-operand tile (one PSUM bank of fp32)
QT = 512        # query tile in phase C
SB = SK // NT   # 2 token blocks per core
WARM = 22       # PE warm-up matmuls (cover the initial DMA window)

SCALE = 1.0 / float(np.sqrt(np.float32(D)))
Identity = mybir.ActivationFunctionType.Identity
Exp = mybir.ActivationFunctionType.Exp


def build_program() -> bass.Bass:
    nc = bacc.Bacc(
        "TRN2", target_bir_lowering=False, debug=False, num_devices=8)

    xT_d = nc.dram_tensor("xT", [SB, P, DC, NT], BF16,
                      kind="ExternalInput").ap()
    wq_d = nc.dram_tensor("Wqr", [EC, P, DC, P], BF16, kind="ExternalInput").ap()
    wk_d = nc.dram_tensor("Wkr", [EC, P, DC, P], BF16, kind="ExternalInput").ap()
    wv_d = nc.dram_tensor("Wvr", [P, DC, D], BF16, kind="ExternalInput").ap()
    bvx_d = nc.dram_tensor("bvxr", [P, D + 2 * EC], F32,
                       kind="ExternalInput").ap()
    o_d = nc.dram_tensor("o_raw", [S, D], BF16, kind="ExternalOutput").ap()
    rs_d = nc.dram_tensor("rs_raw", [S // QT, P, QT // P], F32,
                      kind="ExternalOutput").ap()

    with tile.TileContext(nc) as tc, ExitStack() as ctx:
        const_p = ctx.enter_context(tc.tile_pool(name="const", bufs=1))
        xt_p = ctx.enter_context(tc.tile_pool(name="xt", bufs=SB))
        kt_p = ctx.enter_context(tc.tile_pool(name="kt", bufs=EC))
        qo_p = ctx.enter_context(tc.tile_pool(name="qo", bufs=1))
        qg_p = ctx.enter_context(tc.tile_pool(name="qg", bufs=2))
        tq_p = ctx.enter_context(tc.tile_pool(name="tq", bufs=2))
        v_p = ctx.enter_context(tc.tile_pool(name="v", bufs=KC))
        wq_p = ctx.enter_context(tc.tile_pool(name="wq", bufs=EC))
        wk_p = ctx.enter_context(tc.tile_pool(name="wk", bufs=EC))
        w_p = ctx.enter_context(tc.tile_pool(name="w", bufs=1))
        pt_p = ctx.enter_context(tc.tile_pool(name="ptp", bufs=12))
        osb_p = ctx.enter_context(tc.tile_pool(name="osb", bufs=3))
        st_p = ctx.enter_context(tc.tile_pool(name="stat", bufs=2))
        psA = ctx.enter_context(tc.tile_pool(name="psA", bufs=4, space="PSUM"))
        psB = ctx.enter_context(tc.tile_pool(name="psB", bufs=3, space="PSUM"))
        psR = ctx.enter_context(tc.tile_pool(name="psR", bufs=1, space="PSUM"))
        dram = ctx.enter_context(tc.tile_pool(name="dram", bufs=2,
                                              space="DRAM"))

        qxi = dram.tile([P, EC, SK], BF16)       # own Q^T, collective input
        qxo = dram.tile([2, P, EC, SK], BF16)    # gathered [even, odd]

        # ---- DMA issue order --------------------------------------------
        # Aggregate input bandwidth is ~240GB/s regardless of descriptor
        # shape, so what matters is ARRIVAL ORDER matching consumption
        # order (per-ec weight tiles keep the dependencies fine-grained).
        # Deadline order: xt[0]+wq[0] gate the first chain (~14us); then
        # one wq tile every ~3.4us; xt[1] by ~40us (qb=1 chains); wk by
        # 42..66us; wv/bvb by ~69us.
        xt = [xt_p.tile([P, DC, NT], BF16, name=f"xt{tb}", tag="xt")
              for tb in range(SB)]
        wq = [wq_p.tile([P, DC, P], BF16, name=f"wq{ec}", tag="wq")
              for ec in range(EC)]
        wk = [wk_p.tile([P, DC, P], BF16, name=f"wk{ec}", tag="wk")
              for ec in range(EC)]
        wv = w_p.tile([P, DC, D], BF16, name="wv", tag="w")
        ones_col = const_p.tile([P, 1], BF16)   # lhsT for row-sums
        nc.vector.memset(ones_col[:], 1.0)
        # bv broadcast + bq/bk biases in one fat-lined tensor (a separate
        # [P,16] bias DMA has 64B lines and poisons the queue for ~8us)
        bvx = const_p.tile([P, D + 2 * EC], F32)
        bvb = bvx[:, 0:D]

        # Two queues deliver in deadline order (the scalar HWDGE queue
        # measured ~10-20x slower than sync/gpsimd — never use it; and
        # lines stay >= 2KB — smaller is descriptor-rate-bound and poisons
        # the queue). The qxi exchange DMA and collective ride behind
        # gpsimd's stream (firing when Q drains ~45us).
        nc.gpsimd.dma_start(xt[0][P // 2:P], xT_d[0][P // 2:P])
        nc.gpsimd.dma_start(bvx[P // 2:P], bvx_d[P // 2:P])
        for ec in (1, 3, 5, 7):
            nc.gpsimd.dma_start(wq[ec][:], wq_d[ec])
        nc.gpsimd.dma_start(xt[1][P // 2:P], xT_d[1][P // 2:P])
        for ec in (1, 3, 5, 7):
            nc.gpsimd.dma_start(wk[ec][:], wk_d[ec])
        nc.gpsimd.dma_start(wv[P // 2:P], wv_d[P // 2:P])

        nc.sync.dma_start(xt[0][0:P // 2], xT_d[0][0:P // 2])
        for ec in (0, 2, 4, 6):
            nc.sync.dma_start(wq[ec][:], wq_d[ec])
        nc.sync.dma_start(xt[1][0:P // 2], xT_d[1][0:P // 2])
        nc.sync.dma_start(bvx[0:P // 2], bvx_d[0:P // 2])
        for ec in (0, 2, 4, 6):
            nc.sync.dma_start(wk[ec][:], wk_d[ec])
        nc.sync.dma_start(wv[0:P // 2], wv_d[0:P // 2])

        # PE warm-up during the initial DMA window: dummy matmuls on a
        # memset scratch tile get the HAM clock gate to K=8/8 before the
        # first real chain. Every PSUM group gets a reader (narrow copy) —
        # matmul groups with no consumer have wedged the device.
        scr = const_p.tile([P, NT], BF16)
        nc.vector.memset(scr[:], 0.0)
        scr_out = const_p.tile([P, NT], F32)
        for i in range(WARM):
            pool = psA if i % 2 == 0 else psB
            ps = pool.tile([P, NT], F32)
            nc.tensor.matmul(
                ps[:], scr[:, 0:P], scr[:], start=True, stop=True)
            j = i % 8
            nc.scalar.activation(
                scr_out[:, j * 64:(j + 1) * 64], ps[:, 0:64], Identity)

        # ---- Phase Q: Q^T (own tokens) + exchange ------------------------
        # qb OUTER: the first chain needs only xt[0]+wq[0] (1.25MB), and
        # each later chain adds one 256KB wq tile — matches the DMA stream.
        qown = qo_p.tile([P, EC, SK], BF16, name="qown", tag="qo")
        for qb in range(SB):
            for ec in range(EC):
                ps = psA.tile([P, NT], F32)
                for dc in range(DC):
                    nc.tensor.matmul(
                        ps[:],
                        wq[ec][:, dc, :],
                        xt[qb][:, dc, :],
                        start=(dc == 0), stop=(dc == DC - 1),
                    )
                nc.scalar.activation(
                    qown[:, ec, qb * NT:(qb + 1) * NT], ps[:], Identity,
                    bias=bvx[:, D + ec:D + ec + 1],
                )
        nc.gpsimd.dma_start(qxi[:], qown[:])

        nc.gpsimd.collective_compute(
            "AllGather",
            mybir.AluOpType.bypass,
            replica_groups=[[0, 1], [2, 3], [4, 5], [6, 7]],
            ins=[qxi.opt()],
            outs=[qxo.opt()],
        )

        # ---- Phase K: K^T (own keys) resident ---------------------------
        kt = [kt_p.tile([P, SK], BF16, name=f"kt{ec}", tag="kt")
              for ec in range(EC)]
        for kb in range(SB):
            for ec in range(EC):
                ps = psA.tile([P, NT], F32)
                for dc in range(DC):
                    nc.tensor.matmul(
                        ps[:],
                        wk[ec][:, dc, :],
                        xt[kb][:, dc, :],
                        start=(dc == 0), stop=(dc == DC - 1),
                    )
                nc.scalar.activation(
                    kt[ec][:, kb * NT:(kb + 1) * NT], ps[:], Identity,
                    bias=bvx[:, D + EC + ec:D + EC + ec + 1],
                )

        # ---- Phase V: V natural [k, e] (own keys) resident --------------
        v = [v_p.tile([P, D], BF16, name=f"v{kc}", tag="v") for kc in range(KC)]
        for kc in range(KC):
            for et in range(D // NT):
                ps = psB.tile([P, NT], F32)
                tb, j = divmod(kc, NT // P)
                for dc in range(DC):
                    nc.tensor.matmul(
                        ps[:],
                        xt[tb][:, dc, j * P:(j + 1) * P],
                        wv[:, dc, et * NT:(et + 1) * NT],
                        start=(dc == 0), stop=(dc == DC - 1),
                    )
                # fused bias add on evacuation: v = ps + bvb
                nc.vector.scalar_tensor_tensor(
                    v[kc][:, et * NT:(et + 1) * NT], ps[:], 1.0,
                    bvb[:, et * NT:(et + 1) * NT],
                    mybir.AluOpType.mult, mybir.AluOpType.add,
                )

        # ---- Gather readback --------------------------------------------
        # qx0/qx1 = the two rank slots (identical on both cores). Partner
        # half = (qx0 + qx1) - qown with an fp32 intermediate, so the
        # reconstruction is bit-exact (sum of two bf16 is exact in fp32).
        # Readback rides gpsimd (slot 0) and sync (slot 1); the DVE
        # reconstruct is emitted mid-phase-C (after qq=0's evacuations) so
        # a late collective can't head-of-line-block the O evacuations.
        qx0 = qg_p.tile([P, EC, SK], BF16, name="qx0", tag="qg")
        qx1 = qg_p.tile([P, EC, SK], BF16, name="qx1", tag="qg")
        nc.gpsimd.dma_start(qx0[:], qxo[0])
        nc.sync.dma_start(qx1[:], qxo[1])
        qoth = qx1  # reconstructed in place

        # ---- Phase C: attention, transposed scores ----------------------
        # Query blocks in "rolled" order: qq 0-1 = own tokens (overlap the
        # collective), qq 2-3 = partner tokens via qoth.
        for qq in range(S // QT):
            if qq == 1:
                # exact partner reconstruct on DVE, off the critical path
                for ec in range(EC):
                    tmp = tq_p.tile([P, SK], F32, name="tq", tag="tq")
                    nc.vector.tensor_tensor(
                        tmp[:], qx0[:, ec, :], qx1[:, ec, :],
                        mybir.AluOpType.add)
                    nc.vector.tensor_tensor(
                        qoth[:, ec, :], tmp[:], qown[:, ec, :],
                        mybir.AluOpType.subtract)
            if qq < 2:
                qsrc, qo0 = qown, qq * QT
            else:
                qsrc, qo0 = qoth, (qq - 2) * QT
            # S^T[k, q] per key chunk; exp writes P^T straight to SBUF bf16
            ptt = [pt_p.tile([P, QT], BF16, tag="ptp", name=f"ptt{kc}")
                   for kc in range(KC)]
            for kc in range(KC):
                ps = psA.tile([P, QT], F32)
                for ec in range(EC):
                    nc.tensor.matmul(
                        ps[:],
                        kt[ec][:, kc * P:(kc + 1) * P],
                        qsrc[:, ec, qo0:qo0 + QT],
                        start=(ec == 0), stop=(ec == EC - 1),
                    )
                nc.scalar.activation(ptt[kc][:], ps[:], Exp, scale=SCALE)

            # O~ = P^T.T @ V per 128-query chunk, with the partial row-sum
            # fused in: after each et=0 matmul, an N=1 matmul with the SAME
            # stationary P^T block and a moving ones-column accumulates
            # rs[q] — its LDWEIGHTS hides under the 512-wide stream.
            rs_cols = st_p.tile([P, QT // P], F32, name="rsc", tag="rs")
            last_qq = qq == S // QT - 1
            for qc in range(QT // P):
                last_blk = last_qq and qc == QT // P - 1
                o_sb = osb_p.tile([P, D], BF16, name="osb", tag="osb")
                psq = psR.tile([P, 1], F32, name="psq")
                row0 = qq * QT + qc * P
                for et in range(D // NT):
                    ps = psB.tile([P, NT], F32)
                    for kc in range(KC):
                        nc.tensor.matmul(
                            ps[:],
                            ptt[kc][:, qc * P:(qc + 1) * P],
                            v[kc][:, et * NT:(et + 1) * NT],
                            start=(kc == 0), stop=(kc == KC - 1),
                        )
                        if et == 0:
                            nc.tensor.matmul(
                                psq[:],
                                ptt[kc][:, qc * P:(qc + 1) * P],
                                ones_col[:],
                                start=(kc == 0), stop=(kc == KC - 1),
                            )
                    nc.vector.tensor_copy(
                        o_sb[:, et * NT:(et + 1) * NT], ps[:])
                    if last_blk:
                        # drain the kernel tail: ship each half as soon as
                        # it's evacuated, with the tiny rs DMA in between
                        if et == 0:
                            nc.vector.tensor_copy(
                                rs_cols[:, qc:qc + 1], psq[:])
                            nc.sync.dma_start(
                                o_d[row0:row0 + P, 0:NT], o_sb[:, 0:NT])
                            nc.sync.dma_start(rs_d[qq], rs_cols[:])
                        else:
                            nc.sync.dma_start(
                                o_d[row0:row0 + P, NT:D], o_sb[:, NT:D])
                if not last_blk:
                    nc.vector.tensor_copy(rs_cols[:, qc:qc + 1], psq[:])
                    nc.sync.dma_start(o_d[row0:row0 + P, :], o_sb[:])
            if not last_qq:
                nc.sync.dma_start(rs_d[qq], rs_cols[:])

    nc.compile()
    return nc


_CACHE: dict = {}


def _get_program() -> bass.Bass:
    if "nc" not in _CACHE:
        _CACHE["nc"] = build_program()
    return _CACHE["nc"]


def kernel(x, Wq, bq, Wk, bk, Wv, bv, _trace=False, _trace_kwargs=None):
    nc = _get_program()
    x = np.asarray(x, dtype=np.float32)

    def wrearr(w):
        # [d, e] -> [ec, p(d%128), dc, e%128] so each ec-block DMA is
        # contiguous with 2KB per partition line
        w = np.asarray(w, np.float32).astype(NPBF16)
        return np.ascontiguousarray(
            w.reshape(DC, P, EC, P).transpose(2, 1, 0, 3))

    bvx = np.concatenate([
        np.broadcast_to(np.asarray(bv, np.float32), (P, D)),
        np.asarray(bq, np.float32).reshape(EC, P).T,
        np.asarray(bk, np.float32).reshape(EC, P).T,
    ], axis=1)
    shared = {
        "Wqr": wrearr(Wq),
        "Wkr": wrearr(Wk),
        "Wvr": np.ascontiguousarray(
            np.asarray(Wv, np.float32).astype(NPBF16)
            .reshape(DC, P, D).transpose(1, 0, 2)),
        "bvxr": np.ascontiguousarray(bvx),
    }
    in_maps = []
    for c in range(8):
        b, h = divmod(c, 2)
        xb = x[b][h * SK:(h + 1) * SK]      # own token half only
        xTb = xb.T.astype(NPBF16)           # [D, SK]
        # [d, s] -> [tb, p, dc, t]: each 512-token block fully contiguous
        xTr = np.ascontiguousarray(
            xTb.reshape(DC, P, SB, NT).transpose(2, 1, 0, 3))
        in_maps.append({"xT": xTr, **shared})

    res = run_bass_kernel_spmd(
        nc, in_maps, list(range(8)),
        trace=_trace, **(_trace_kwargs or {}),
    )
    out = np.empty((4, S, D), dtype=np.float32)
    for b in range(4):
        def unrs(r):
            # device writes rs as [qq, p, qc] = rowsum(qq*512 + qc*128 + p)
            return r.reshape(S // QT, P, QT // P).transpose(
                0, 2, 1).reshape(S).astype(np.float64)

        o0 = res.results[2 * b]["o_raw"].astype(np.float64)
        r0 = unrs(res.results[2 * b]["rs_raw"])
        o1 = res.results[2 * b + 1]["o_raw"].astype(np.float64)
        r1 = unrs(res.results[2 * b + 1]["rs_raw"])
        # core h=1 computed queries in rolled order; un-roll before combining
        o1 = np.roll(o1, SK, axis=0)
        r1 = np.roll(r1, SK)
        out[b] = ((o0 + o1) / (r0 + r1)[:, None]).astype(np.float32)
    if _trace:
        return out, res
    return out


# revision 30
# speedup vs baseline: 1.2734x; 1.0090x over previous
"""Single-head attention (B=4, S=2048, D=1024) on 8 Trainium2 NeuronCores.

Sharding: batch x KEY-half with a pairwise Q exchange. Core c handles batch
b=c//2 and token half h=c%2 (its 1024 "own" tokens are both its keys and the
first half of its query order). Each core computes K/V/Q for its OWN 1024
tokens only (no duplicated projection work), then the two cores of a batch
exchange Q via an in-kernel AllGather (replica groups {0,1},{2,3},...). Each
core then computes the UNNORMALIZED partial attention O~ = exp(S)V for ALL
2048 queries against its 1024 keys, plus partial row-sums; the host combines
the pair: O = (O~_0 + O~_1) / (rs_0 + rs_1).

The AllGather output is rank-ordered ([Q_even_core, Q_odd_core]) — identical
on both cores — but each core needs the PARTNER half. To stay SPMD (no
per-core addressing), the partner half is reconstructed exactly as
qoth = (qx0 + qx1) - qown on DVE with an fp32 intermediate (a sum of two
bf16 is exact in fp32, so subtracting qown returns the partner's bf16 bits).
Queries are processed own-half-first ("rolled" order), so the first half of
phase C overlaps the collective; the host un-rolls odd cores' outputs.

All matmul operands are bf16 (PSUM accumulation stays fp32); per-core PE work
is 15.0 GFLOP (vs 17.2 without the exchange). Every bulk tensor is
host-arranged partition-major so it loads as ONE DMA with 16KB contiguous
lines per partition (the 2KB-line per-ec layout was descriptor-rate-bound
and starved phase Q). A dummy-matmul warmup keeps the PE busy through the
initial DMA window so the HAM clock gate reaches 8/8 before the first real
chain.

Per-core pipeline (activations kept [feature, token] transposed so the PE
contracts over partitions):
  Q:  Q^T[ec] = Wq[:,ec-blk].T @ x^T  (own 1024 tokens; DMA'd out as one
      block, AllGather'd during phases K/V)
  K:  K^T[ec] = Wk[:,ec-blk].T @ x^T
  V:  V[kc]   = x^T[:, kc-blk].T @ Wv  (bias via host-sent broadcast tile)
  C:  per 512-query block: S^T[k,q] = K^T.T @ Q^T; exp on ACT writes P^T
      straight to SBUF as bf16; partial row-sums via ones-vector matmuls
      fused into the O chain; O~ = P^T.T @ V; DMA out raw.
"""

import sys
from contextlib import ExitStack

import ml_dtypes
import numpy as np

if "/opt/trn_rl_repo" not in sys.path:
    sys.path.insert(0, "/opt/trn_rl_repo")

import concourse.bass as bass
import concourse.bacc as bacc
import concourse.tile as tile
from concourse import mybir
from concourse.bass_utils import run_bass_kernel_spmd

P = 128
S = 2048        # full sequence (queries per core)
SK = 1024       # own tokens per core (keys; also own query half)
D = 1024        # model dim
F32 = mybir.dt.float32
BF16 = mybir.dt.bfloat16
NPBF16 = ml_dtypes.bfloat16

DC = D // P     # 8 d-chunks (contraction over model dim)
EC = D // P     # 8 e-chunks (output features)
KC = SK // P    # 8 key chunks (own half)
NT = 512        # moving-operand tile (one PSUM bank of fp32)
QT = 512        # query tile in phase C
SB = SK // NT   # 2 token blocks per core
WARM = 12       # dense PE warm-up matmuls (plus data-gated stragglers)

SCALE = 1.0 / float(np.sqrt(np.float32(D)))
Identity = mybir.ActivationFunctionType.Identity
Exp = mybir.ActivationFunctionType.Exp


def build_program() -> bass.Bass:
    nc = bacc.Bacc(
        "TRN2", target_bir_lowering=False, debug=False, num_devices=8)

    xT_d = nc.dram_tensor("xT", [SB, P, DC, NT], BF16,
                      kind="ExternalInput").ap()
    wq_d = nc.dram_tensor("Wqr", [EC, P, DC, P], BF16, kind="ExternalInput").ap()
    wk_d = nc.dram_tensor("Wkr", [EC, P, DC, P], BF16, kind="ExternalInput").ap()
    wv_d = nc.dram_tensor("Wvr", [P, DC, D], BF16, kind="ExternalInput").ap()
    bvx_d = nc.dram_tensor("bvxr", [P, D + 2 * EC], F32,
                       kind="ExternalInput").ap()
    o_d = nc.dram_tensor("o_raw", [S, D], BF16, kind="ExternalOutput").ap()
    rs_d = nc.dram_tensor("rs_raw", [S // QT, P, QT // P], F32,
                      kind="ExternalOutput").ap()

    with tile.TileContext(nc) as tc, ExitStack() as ctx:
        const_p = ctx.enter_context(tc.tile_pool(name="const", bufs=1))
        xt_p = ctx.enter_context(tc.tile_pool(name="xt", bufs=SB))
        kt_p = ctx.enter_context(tc.tile_pool(name="kt", bufs=EC))
        qo_p = ctx.enter_context(tc.tile_pool(name="qo", bufs=1))
        qg_p = ctx.enter_context(tc.tile_pool(name="qg", bufs=2))
        tq_p = ctx.enter_context(tc.tile_pool(name="tq", bufs=2))
        v_p = ctx.enter_context(tc.tile_pool(name="v", bufs=KC))
        wq_p = ctx.enter_context(tc.tile_pool(name="wq", bufs=EC))
        wk_p = ctx.enter_context(tc.tile_pool(name="wk", bufs=EC))
        w_p = ctx.enter_context(tc.tile_pool(name="w", bufs=1))
        pt_p = ctx.enter_context(tc.tile_pool(name="ptp", bufs=12))
        osb_p = ctx.enter_context(tc.tile_pool(name="osb", bufs=3))
        st_p = ctx.enter_context(tc.tile_pool(name="stat", bufs=2))
        psA = ctx.enter_context(tc.tile_pool(name="psA", bufs=4, space="PSUM"))
        psB = ctx.enter_context(tc.tile_pool(name="psB", bufs=3, space="PSUM"))
        psR = ctx.enter_context(tc.tile_pool(name="psR", bufs=1, space="PSUM"))
        dram = ctx.enter_context(tc.tile_pool(name="dram", bufs=2,
                                              space="DRAM"))

        qxi = dram.tile([P, EC, SK], BF16)       # own Q^T, collective input
        qxo = dram.tile([2, P, EC, SK], BF16)    # gathered [even, odd]

        # ---- DMA issue order --------------------------------------------
        # Aggregate input bandwidth is ~240GB/s regardless of descriptor
        # shape, so what matters is ARRIVAL ORDER matching consumption
        # order (per-ec weight tiles keep the dependencies fine-grained).
        # Deadline order: xt[0]+wq[0] gate the first chain (~14us); then
        # one wq tile every ~3.4us; xt[1] by ~40us (qb=1 chains); wk by
        # 42..66us; wv/bvb by ~69us.
        xt = [xt_p.tile([P, DC, NT], BF16, name=f"xt{tb}", tag="xt")
              for tb in range(SB)]
        wq = [wq_p.tile([P, DC, P], BF16, name=f"wq{ec}", tag="wq")
              for ec in range(EC)]
        wk = [wk_p.tile([P, DC, P], BF16, name=f"wk{ec}", tag="wk")
              for ec in range(EC)]
        wv = w_p.tile([P, DC, D], BF16, name="wv", tag="w")
        ones_col = const_p.tile([P, 1], BF16)   # lhsT for row-sums
        nc.vector.memset(ones_col[:], 1.0)
        # bv broadcast + bq/bk biases in one fat-lined tensor (a separate
        # [P,16] bias DMA has 64B lines and poisons the queue for ~8us)
        bvx = const_p.tile([P, D + 2 * EC], F32)
        bvb = bvx[:, 0:D]

        # Two queues deliver in deadline order (the scalar HWDGE queue
        # measured ~10-20x slower than sync/gpsimd — never use it; and
        # lines stay >= 2KB — smaller is descriptor-rate-bound and poisons
        # the queue). The qxi exchange DMA and collective ride behind
        # gpsimd's stream (firing when Q drains ~45us).
        nc.gpsimd.dma_start(xt[0][P // 2:P], xT_d[0][P // 2:P])
        nc.gpsimd.dma_start(bvx[P // 2:P], bvx_d[P // 2:P])
        for ec in (1, 3, 5, 7):
            nc.gpsimd.dma_start(wq[ec][:], wq_d[ec])
        nc.gpsimd.dma_start(xt[1][P // 2:P], xT_d[1][P // 2:P])
        for ec in (1, 3, 5, 7):
            nc.gpsimd.dma_start(wk[ec][:], wk_d[ec])
        nc.gpsimd.dma_start(wv[P // 2:P], wv_d[P // 2:P])

        nc.sync.dma_start(xt[0][0:P // 2], xT_d[0][0:P // 2])
        for ec in (0, 2, 4, 6):
            nc.sync.dma_start(wq[ec][:], wq_d[ec])
        nc.sync.dma_start(xt[1][0:P // 2], xT_d[1][0:P // 2])
        nc.sync.dma_start(bvx[0:P // 2], bvx_d[0:P // 2])
        for ec in (0, 2, 4, 6):
            nc.sync.dma_start(wk[ec][:], wk_d[ec])
        nc.sync.dma_start(wv[0:P // 2], wv_d[0:P // 2])

        # PE warm-up during the initial DMA window: dummy matmuls on a
        # memset scratch tile get the HAM clock gate to K=8/8 before the
        # first real chain. Every PSUM group gets a reader (narrow copy) —
        # matmul groups with no consumer have wedged the device.
        scr = const_p.tile([P, NT], BF16)
        nc.vector.memset(scr[:], 0.0)
        scr_out = const_p.tile([P, NT], F32)
        warm_lhs = ([scr[:, 0:P]] * WARM
                    + [xt[0][:, dc, 0:P] for dc in range(4)]
                    + [wq[0][:, dc, :] for dc in range(2)])
        for i, lhs in enumerate(warm_lhs):
            # the tail entries are gated on the first real tiles landing:
            # they re-tick the PE at data-arrival granularity so no idle
            # gap exceeds the ~3.4us HAM MID window and the first real
            # chain runs at full clock
            pool = psA if i % 2 == 0 else psB
            ps = pool.tile([P, NT], F32)
            nc.tensor.matmul(ps[:], lhs, scr[:], start=True, stop=True)
            j = i % 8
            nc.scalar.activation(
                scr_out[:, j * 64:(j + 1) * 64], ps[:, 0:64], Identity)

        # ---- Phase Q: Q^T (own tokens) + exchange ------------------------
        # qb OUTER: the first chain needs only xt[0]+wq[0] (1.25MB), and
        # each later chain adds one 256KB wq tile — matches the DMA stream.
        qown = qo_p.tile([P, EC, SK], BF16, name="qown", tag="qo")
        for qb in range(SB):
            for ec in range(EC):
                ps = psA.tile([P, NT], F32)
                for dc in range(DC):
                    nc.tensor.matmul(
                        ps[:],
                        wq[ec][:, dc, :],
                        xt[qb][:, dc, :],
                        start=(dc == 0), stop=(dc == DC - 1),
                    )
                nc.scalar.activation(
                    qown[:, ec, qb * NT:(qb + 1) * NT], ps[:], Identity,
                    bias=bvx[:, D + ec:D + ec + 1],
                )
        nc.gpsimd.dma_start(qxi[:], qown[:])

        nc.gpsimd.collective_compute(
            "AllGather",
            mybir.AluOpType.bypass,
            replica_groups=[[0, 1], [2, 3], [4, 5], [6, 7]],
            ins=[qxi.opt()],
            outs=[qxo.opt()],
        )

        # ---- Phase K: K^T (own keys) resident ---------------------------
        kt = [kt_p.tile([P, SK], BF16, name=f"kt{ec}", tag="kt")
              for ec in range(EC)]
        for kb in range(SB):
            for ec in range(EC):
                ps = psA.tile([P, NT], F32)
                for dc in range(DC):
                    nc.tensor.matmul(
                        ps[:],
                        wk[ec][:, dc, :],
                        xt[kb][:, dc, :],
                        start=(dc == 0), stop=(dc == DC - 1),
                    )
                nc.scalar.activation(
                    kt[ec][:, kb * NT:(kb + 1) * NT], ps[:], Identity,
                    bias=bvx[:, D + EC + ec:D + EC + ec + 1],
                )

        # ---- Phase V: V natural [k, e] (own keys) resident --------------
        v = [v_p.tile([P, D], BF16, name=f"v{kc}", tag="v") for kc in range(KC)]
        for kc in range(KC):
            for et in range(D // NT):
                ps = psB.tile([P, NT], F32)
                tb, j = divmod(kc, NT // P)
                for dc in range(DC):
                    nc.tensor.matmul(
                        ps[:],
                        xt[tb][:, dc, j * P:(j + 1) * P],
                        wv[:, dc, et * NT:(et + 1) * NT],
                        start=(dc == 0), stop=(dc == DC - 1),
                    )
                # fused bias add on evacuation: v = ps + bvb
                nc.vector.scalar_tensor_tensor(
                    v[kc][:, et * NT:(et + 1) * NT], ps[:], 1.0,
                    bvb[:, et * NT:(et + 1) * NT],
                    mybir.AluOpType.mult, mybir.AluOpType.add,
                )

        # ---- Gather readback --------------------------------------------
        # qx0/qx1 = the two rank slots (identical on both cores). Partner
        # half = (qx0 + qx1) - qown with an fp32 intermediate, so the
        # reconstruction is bit-exact (sum of two bf16 is exact in fp32).
        # Readback rides gpsimd (slot 0) and sync (slot 1); the DVE
        # reconstruct is emitted mid-phase-C (after qq=0's evacuations) so
        # a late collective can't head-of-line-block the O evacuations.
        qx0 = qg_p.tile([P, EC, SK], BF16, name="qx0", tag="qg")
        qx1 = qg_p.tile([P, EC, SK], BF16, name="qx1", tag="qg")
        nc.gpsimd.dma_start(qx0[:], qxo[0])
        nc.sync.dma_start(qx1[:], qxo[1])
        qoth = qx1  # reconstructed in place

        # ---- Phase C: attention, transposed scores ----------------------
        # Query blocks in "rolled" order: qq 0-1 = own tokens (overlap the
        # collective), qq 2-3 = partner tokens via qoth.
        for qq in range(S // QT):
            if qq == 1:
                # exact partner reconstruct on DVE, off the critical path
                for ec in range(EC):
                    tmp = tq_p.tile([P, SK], F32, name="tq", tag="tq")
                    nc.vector.tensor_tensor(
                        tmp[:], qx0[:, ec, :], qx1[:, ec, :],
                        mybir.AluOpType.add)
                    nc.vector.tensor_tensor(
                        qoth[:, ec, :], tmp[:], qown[:, ec, :],
                        mybir.AluOpType.subtract)
            if qq < 2:
                qsrc, qo0 = qown, qq * QT
            else:
                qsrc, qo0 = qoth, (qq - 2) * QT
            # S^T[k, q] per key chunk; exp writes P^T straight to SBUF bf16
            ptt = [pt_p.tile([P, QT], BF16, tag="ptp", name=f"ptt{kc}")
                   for kc in range(KC)]
            for kc in range(KC):
                ps = psA.tile([P, QT], F32)
                for ec in range(EC):
                    nc.tensor.matmul(
                        ps[:],
                        kt[ec][:, kc * P:(kc + 1) * P],
                        qsrc[:, ec, qo0:qo0 + QT],
                        start=(ec == 0), stop=(ec == EC - 1),
                    )
                nc.scalar.activation(ptt[kc][:], ps[:], Exp, scale=SCALE)

            # O~ = P^T.T @ V per 128-query chunk, with the partial row-sum
            # fused in: after each et=0 matmul, an N=1 matmul with the SAME
            # stationary P^T block and a moving ones-column accumulates
            # rs[q] — its LDWEIGHTS hides under the 512-wide stream.
            rs_cols = st_p.tile([P, QT // P], F32, name="rsc", tag="rs")
            last_qq = qq == S // QT - 1
            for qc in range(QT // P):
                last_blk = last_qq and qc == QT // P - 1
                o_sb = osb_p.tile([P, D], BF16, name="osb", tag="osb")
                psq = psR.tile([P, 1], F32, name="psq")
                row0 = qq * QT + qc * P
                for et in range(D // NT):
                    ps = psB.tile([P, NT], F32)
                    for kc in range(KC):
                        nc.tensor.matmul(
                            ps[:],
                            ptt[kc][:, qc * P:(qc + 1) * P],
                            v[kc][:, et * NT:(et + 1) * NT],
                            start=(kc == 0), stop=(kc == KC - 1),
                        )
                        if et == 0:
                            nc.tensor.matmul(
                                psq[:],
                                ptt[kc][:, qc * P:(qc + 1) * P],
                                ones_col[:],
                                start=(kc == 0), stop=(kc == KC - 1),
                            )
                    nc.vector.tensor_copy(
                        o_sb[:, et * NT:(et + 1) * NT], ps[:])
                    if last_blk:
                        # drain the kernel tail: ship each half as soon as
                        # it's evacuated, with the tiny rs DMA in between
                        if et == 0:
                            nc.vector.tensor_copy(
                                rs_cols[:, qc:qc + 1], psq[:])
                            nc.sync.dma_start(
                                o_d[row0:row0 + P, 0:NT], o_sb[:, 0:NT])
                            nc.sync.dma_start(rs_d[qq], rs_cols[:])
                        else:
                            nc.sync.dma_start(
                                o_d[row0:row0 + P, NT:D], o_sb[:, NT:D])
                if not last_blk:
                    nc.vector.tensor_copy(rs_cols[:, qc:qc + 1], psq[:])
                    nc.sync.dma_start(o_d[row0:row0 + P, :], o_sb[:])
            if not last_qq:
                nc.sync.dma_start(rs_d[qq], rs_cols[:])

    nc.compile()
    return nc


_CACHE: dict = {}


def _get_program() -> bass.Bass:
    if "nc" not in _CACHE:
        _CACHE["nc"] = build_program()
    return _CACHE["nc"]


def kernel(x, Wq, bq, Wk, bk, Wv, bv, _trace=False, _trace_kwargs=None):
    nc = _get_program()
    x = np.asarray(x, dtype=np.float32)

    def wrearr(w):
        # [d, e] -> [ec, p(d%128), dc, e%128] so each ec-block DMA is
        # contiguous with 2KB per partition line
        w = np.asarray(w, np.float32).astype(NPBF16)
        return np.ascontiguousarray(
            w.reshape(DC, P, EC, P).transpose(2, 1, 0, 3))

    bvx = np.concatenate([
        np.broadcast_to(np.asarray(bv, np.float32), (P, D)),
        np.asarray(bq, np.float32).reshape(EC, P).T,
        np.asarray(bk, np.float32).reshape(EC, P).T,
    ], axis=1)
    shared = {
        "Wqr": wrearr(Wq),
        "Wkr": wrearr(Wk),
        "Wvr": np.ascontiguousarray(
            np.asarray(Wv, np.float32).astype(NPBF16)
            .reshape(DC, P, D).transpose(1, 0, 2)),
        "bvxr": np.ascontiguousarray(bvx),
    }
    in_maps = []
    for c in range(8):
        b, h = divmod(c, 2)
        xb = x[b][h * SK:(h + 1) * SK]      # own token half only
        xTb = xb.T.astype(NPBF16)           # [D, SK]
        # [d, s] -> [tb, p, dc, t]: each 512-token block fully contiguous
        xTr = np.ascontiguousarray(
            xTb.reshape(DC, P, SB, NT).transpose(2, 1, 0, 3))
        in_maps.append({"xT": xTr, **shared})

    res = run_bass_kernel_spmd(
        nc, in_maps, list(range(8)),
        trace=_trace, **(_trace_kwargs or {}),
    )
    out = np.empty((4, S, D), dtype=np.float32)
    for b in range(4):
        def unrs(r):
            # device writes rs as [qq, p, qc] = rowsum(qq*512 + qc*128 + p)
            return r.reshape(S // QT, P, QT // P).transpose(
                0, 2, 1).reshape(S).astype(np.float64)

        o0 = res.results[2 * b]["o_raw"].astype(np.float64)
        r0 = unrs(res.results[2 * b]["rs_raw"])
        o1 = res.results[2 * b + 1]["o_raw"].astype(np.float64)
        r1 = unrs(res.results[2 * b + 1]["rs_raw"])
        # core h=1 computed queries in rolled order; un-roll before combining
        o1 = np.roll(o1, SK, axis=0)
        r1 = np.roll(r1, SK)
        out[b] = ((o0 + o1) / (r0 + r1)[:, None]).astype(np.float32)
    if _trace:
        return out, res
    return out
